# revision 13
# baseline (speedup 1.0000x reference)
"""Trainium2 Bass kernel for nn_C2f_DualModal_MoE.

Full inputs in, full outputs out. Data-parallel over batch: 16 items on
8 cores (2 per core). Routing (softmax top-2) computed on device; the two
selected experts' 3x3 conv weights are gathered with a dynamic-offset DMA.
Matmuls run in float32r (1 row/cycle on the PE vs 4 for float32).

Structure per core (items b=0,1):
  p1(0) -> rt(0) -> [p2(0) group-interleaved with p1(1)] -> rt(1) -> p2(1)
Spatial tiling: 16 tiles of 5 image rows (N=400 matmul columns), processed
in groups of 2 with weight-stationary inner loops (halves LDWEIGHTS traffic).
"""

import sys

for _p in ("/opt/trn_rl_repo", "/opt/pypackages"):
    if _p not in sys.path:
        sys.path.append(_p)

import numpy as np
import concourse.bass as bass
import concourse.mybir as mybir
import concourse.tile as tile
from concourse import bacc
from concourse.bass import ds
from concourse.bass_utils import run_bass_kernel_spmd

F32 = mybir.dt.float32
F32R = mybir.dt.float32r
AF = mybir.ActivationFunctionType

N_CORES = 8
B = 16
BPC = B // N_CORES  # batch items per core
C1 = 256  # cv1 input channels
C = 128  # hidden width
E = 4  # experts
H = W = 80
S = H * W  # 6400
R = 5  # image rows per spatial tile
NT = H // R  # 16 spatial tiles
N = R * W  # 400 columns per matmul
G = 2  # spatial tiles per group (weight-stationary)
NG = NT // G  # 8 groups
HP = H + 2  # padded image size (82)
WBLOB = 9 * C  # 1152 expert weight cols
INV_S = 1.0 / S

_cache = {}


def _build_program(reps=1):
    nc = bacc.Bacc(
        "TRN2",
        target_bir_lowering=False,
        debug=False,
        enable_asserts=True,
        dynamic_dma_scratch_size=4096,
    )
    x_d = nc.dram_tensor("x", [BPC, C1, S], F32, kind="ExternalInput").ap()
    w1_d = nc.dram_tensor("w1t", [C1, 2 * C], F32, kind="ExternalInput").ap()
    b1_d = nc.dram_tensor("b1", [2 * C, 1], F32, kind="ExternalInput").ap()
    wr_d = nc.dram_tensor("wrt", [C, E], F32, kind="ExternalInput").ap()
    br_d = nc.dram_tensor("br", [E, 1], F32, kind="ExternalInput").ap()
    wexp_d = nc.dram_tensor("wexp", [E * C, WBLOB + 1], F32, kind="ExternalInput").ap()
    w2_d = nc.dram_tensor("w2t", [3 * C, 2 * C], F32, kind="ExternalInput").ap()
    b2_d = nc.dram_tensor("b2", [2 * C, 1], F32, kind="ExternalInput").ap()
    out_d = nc.dram_tensor("out", [BPC, 2 * C, S], F32, kind="ExternalOutput").ap()

    with tile.TileContext(nc) as tc:
        _emit(nc, tc, x_d, w1_d, b1_d, wr_d, br_d, wexp_d, w2_d, b2_d, out_d, reps)
    nc.compile()
    return nc


def _emit(nc, tc, x_d, w1_d, b1_d, wr_d, br_d, wexp_d, w2_d, b2_d, out_d, reps=1):
    from contextlib import ExitStack

    ctx = ExitStack()
    with ctx:
        wp = ctx.enter_context(tc.tile_pool(name="weights", bufs=1))
        sp = ctx.enter_context(tc.tile_pool(name="stream", bufs=2))
        pp = ctx.enter_context(tc.tile_pool(name="psum", bufs=1, space="PSUM"))

        def load_f32r(name, src, cols):
            """DMA f32 from DRAM into a raw tile, round into an f32r tile."""
            raw = sp.tile([C, cols], F32, tag="wraw", bufs=2, name=f"{name}raw")
            nc.sync.dma_start(raw[:], src)
            t = wp.tile([C, cols], F32R, name=name)
            nc.vector.tensor_copy(t[:], raw[:])
            return t

        # cv1 weights first (critical path to first matmul)
        w1 = [
            load_f32r(f"w1r{k}", w1_d[k * C : (k + 1) * C, :], 2 * C) for k in range(2)
        ]
        b1 = wp.tile([C, 2], F32, name="b1sb")
        for mt in range(2):
            nc.sync.dma_start(b1[:, mt : mt + 1], b1_d[mt * C : (mt + 1) * C, :])
        zrow = wp.tile([C, HP], F32, name="zrow")
        nc.vector.memset(zrow[:], 0.0)

        def setup_tail():
            w2 = []
            for k in range(2):
                raw = sp.tile([C, 2 * C], F32, tag="wraw", bufs=2, name=f"w2r{k}raw")
                nc.gpsimd.dma_start(raw[:], w2_d[k * C : (k + 1) * C, :])
                t = wp.tile([C, 2 * C], F32R, name=f"w2r{k}")
                nc.vector.tensor_copy(t[:], raw[:])
                w2.append(t)
            w2moe_raw = wp.tile([C, 2 * C], F32, name="w2moeraw")
            nc.gpsimd.dma_start(w2moe_raw[:], w2_d[2 * C : 3 * C, :])
            b2 = wp.tile([C, 2], F32, name="b2sb")
            for mt in range(2):
                nc.gpsimd.dma_start(b2[:, mt : mt + 1], b2_d[mt * C : (mt + 1) * C, :])
            wrt = wp.tile([C, E], F32, name="wrtsb")
            nc.gpsimd.dma_start(wrt[:], wr_d[:])
            br = wp.tile([E, 1], F32, name="brsb")
            nc.gpsimd.dma_start(br[:], br_d[:])
            ones = wp.tile([1, C], F32, name="ones")
            nc.vector.memset(ones[:], 1.0)
            return w2, w2moe_raw, b2, wrt, br, ones

        PTAGS = ("cv", "exp")

        def p1_group(b, g, st, ptag=None):
            """cv1 for spatial tiles [g*G, (g+1)*G)."""
            a_sb, m_pad, parts = st
            if ptag is None:
                ptag = PTAGS[g % 2]
            xr = []
            for k in range(2):
                xcs = []
                for i in range(G):
                    t = g * G + i
                    xw = sp.tile([C, N], F32, tag=f"xw{k}", bufs=4, name=f"xw{k}_{i}")
                    nc.sync.dma_start(
                        xw[:], x_d[b, k * C : (k + 1) * C, t * N : (t + 1) * N]
                    )
                    xc = sp.tile([C, N], F32R, tag=f"x{k}", bufs=4, name=f"xc{k}_{i}")
                    nc.vector.tensor_copy(xc[:], xw[:])
                    xcs.append(xc)
                xr.append(xcs)
            pss = [[None] * G, [None] * G]
            for k in range(2):
                for mt in range(2):
                    ms = slice(mt * C, (mt + 1) * C)
                    for i in range(G):
                        if k == 0:
                            pss[mt][i] = pp.tile([C, N], F32, tag=ptag, bufs=4, name=f"ps1_{mt}_{i}")
                        nc.tensor.matmul(
                            pss[mt][i][:],
                            w1[k][:, ms],
                            xr[k][i][:],
                            start=(k == 0),
                            stop=(k == 1),
                        )
            for i in range(G):
                t = g * G + i
                if i % 2 == 0:
                    nc.vector.tensor_copy(a_sb[:, t * N : (t + 1) * N], pss[0][i][:])
                else:
                    nc.scalar.activation(
                        a_sb[:, t * N : (t + 1) * N], pss[0][i][:], AF.Copy
                    )
                nc.scalar.activation(
                    m_pad[:, 1 + t * R : 1 + (t + 1) * R, 1 : 1 + W],
                    pss[1][i][:],
                    AF.Silu,
                    bias=b1[:, 1:2],
                    accum_out=parts[:, t : t + 1],
                )

        def p1_state(b):
            a_sb = sp.tile([C, S], F32, tag="a", bufs=2)
            m_pad = sp.tile([C, HP, HP], F32R, tag="mpad", bufs=2)
            parts = sp.tile([C, NT], F32, tag="parts", bufs=2)
            nc.vector.tensor_copy(m_pad[:, 0:1, :], zrow[:, None, :])
            nc.vector.tensor_copy(m_pad[:, HP - 1 : HP, :], zrow[:, None, :])
            nc.vector.tensor_copy(m_pad[:, 1 : HP - 1, 0:1], zrow[:, 0 : HP - 2, None])
            nc.vector.tensor_copy(
                m_pad[:, 1 : HP - 1, HP - 1 : HP], zrow[:, 0 : HP - 2, None]
            )
            return a_sb, m_pad, parts

        def routing(b, parts):
            pooled = sp.tile([C, 1], F32, tag="pooled", bufs=2)
            nc.vector.reduce_sum(pooled[:], parts[:], axis=mybir.AxisListType.X)
            ps_l = pp.tile([E, 1], F32, tag="cv", bufs=4)
            nc.tensor.matmul(ps_l[:], wrt[:], pooled[:], start=True, stop=True)
            # logits (mean-scaled) + bias, all off the ACT engine
            l_sb = sp.tile([E, 1], F32, tag="lsb", bufs=2)
            nc.vector.tensor_scalar(
                l_sb[:], ps_l[:], INV_S, None, op0=mybir.AluOpType.mult
            )
            nc.vector.tensor_tensor(l_sb[:], l_sb[:], br[:], op=mybir.AluOpType.add)
            row = sp.tile([1, 8], F32, tag="row", bufs=2)
            nc.vector.memset(row[:], -1e30)
            nc.gpsimd.dma_start(row[0:1, 0:E], l_sb[0:E, 0:1])
            vals = sp.tile([1, 8], F32, tag="vals", bufs=2)
            nc.vector.max(vals[:], row[:])
            uidx = sp.tile([1, 8], mybir.dt.uint32, tag="uidx", bufs=2)
            nc.vector.max_index(uidx[:], vals[:], row[:])
            # gates: g0 = sigmoid(l0 - l1) = silu(d)/d, g1 = 1 - g0
            scr = sp.tile([1, 4], F32, tag="scr", bufs=2)
            nc.vector.tensor_tensor(
                scr[:, 0:1], vals[:, 0:1], vals[:, 1:2], op=mybir.AluOpType.subtract
            )
            nc.vector.reciprocal(scr[:, 1:2], scr[:, 0:1])
            nc.scalar.activation(scr[:, 2:3], scr[:, 0:1], AF.Silu)
            g = sp.tile([1, 2], F32, tag="g", bufs=2)
            nc.vector.tensor_tensor(
                g[:, 0:1], scr[:, 2:3], scr[:, 1:2], op=mybir.AluOpType.mult
            )
            nc.vector.tensor_scalar(
                g[:, 1:2],
                g[:, 0:1],
                -1.0,
                1.0,
                op0=mybir.AluOpType.mult,
                op1=mybir.AluOpType.add,
            )
            ps_g = pp.tile([C, 2], F32, tag="cv", bufs=4)
            nc.tensor.matmul(ps_g[:], ones[:], g[:], start=True, stop=True)
            g_bc = sp.tile([C, 2], F32, tag="gbc", bufs=2)
            nc.vector.tensor_copy(g_bc[:], ps_g[:])
            wks, w2ms = [], []
            for k in range(2):
                iv = nc.values_load(
                    uidx[0:1, k : k + 1],
                    min_val=0,
                    max_val=E - 1,
                    skip_runtime_bounds_check=True,
                )
                raw = sp.tile([C, WBLOB + 1], F32, tag="expraw", bufs=1, name="expraw")
                nc.gpsimd.dma_start(raw[:], wexp_d[ds(iv * C, C), :])
                wk = sp.tile([C, WBLOB], F32R, tag=f"expw{k}", bufs=2, name=f"expw{k}")
                nc.vector.tensor_copy(wk[:], raw[:, 0:WBLOB])
                bk = sp.tile([C, 1], F32, tag=f"expb{k}", bufs=2, name=f"expb{k}")
                nc.vector.tensor_copy(bk[:], raw[:, WBLOB : WBLOB + 1])
                w2m = sp.tile([C, 2 * C], F32R, tag=f"w2m{k}", bufs=2)
                nc.vector.tensor_scalar_mul(w2m[:], w2moe_raw[:], g_bc[:, k : k + 1])
                wks.append((wk, bk))
                w2ms.append(w2m)
            return wks, w2ms

        def p2_group(b, g, st, rt):
            a_sb, m_pad, _ = st
            wks, w2ms = rt
            r0s = [(g * G + i) * R for i in range(G)]
            sa = []
            for i in range(G):
                t = g * G + i
                sai = sp.tile([C, N], F32R, tag="sa", bufs=2, name=f"sa{i}")
                nc.scalar.activation(
                    sai[:], a_sb[:, t * N : (t + 1) * N], AF.Silu, bias=b1[:, 0:1]
                )
                sa.append(sai)
            s_tiles = [[None, None] for _ in range(G)]
            for k in range(2):
                pse = [pp.tile([C, N], F32, tag="exp", bufs=4, name=f"pse{k}_{i}") for i in range(G)]
                for tap in range(9):
                    dy, dx = tap // 3, tap % 3
                    for i in range(G):
                        nc.tensor.matmul(
                            pse[i][:],
                            wks[k][0][:, tap * C : (tap + 1) * C],
                            m_pad[:, r0s[i] + dy : r0s[i] + dy + R, dx : dx + W],
                            start=(tap == 0),
                            stop=(tap == 8),
                        )
                for i in range(G):
                    sk = sp.tile([C, N], F32R, tag=f"s{k}", bufs=2)
                    nc.scalar.activation(
                        sk[:], pse[i][:], AF.Silu, bias=wks[k][1][:]
                    )
                    s_tiles[i][k] = sk
            oss = [[None] * G, [None] * G]  # [mt][i]
            for mt in range(2):
                for i in range(G):
                    oss[mt][i] = pp.tile([C, N], F32, tag="cv", bufs=4, name=f"pso_{mt}_{i}")
            chunks = [
                (w2[0], lambda i: sa[i][:]),
                (w2[1], lambda i: m_pad[:, r0s[i] + 1 : r0s[i] + 1 + R, 1 : 1 + W]),
                (w2ms[0], lambda i: s_tiles[i][0][:]),
                (w2ms[1], lambda i: s_tiles[i][1][:]),
            ]
            for ci, (wt, rhs) in enumerate(chunks):
                for mt in range(2):
                    ms = slice(mt * C, (mt + 1) * C)
                    for i in range(G):
                        nc.tensor.matmul(
                            oss[mt][i][:],
                            wt[:, ms],
                            rhs(i),
                            start=(ci == 0),
                            stop=(ci == 3),
                        )
            for mt in range(2):
                ms = slice(mt * C, (mt + 1) * C)
                for i in range(G):
                    ot = sp.tile([C, N], F32, tag=f"ot{mt}", bufs=2, name=f"ot{mt}_{i}")
                    nc.scalar.activation(
                        ot[:], oss[mt][i][:], AF.Silu, bias=b2[:, mt : mt + 1]
                    )
                    t = g * G + i
                    eng = nc.sync if (mt == 0 or g == NG - 1) else nc.gpsimd
                    eng.dma_start(out_d[b, ms, t * N : (t + 1) * N], ot[:])

        w2 = w2moe_raw = b2 = wrt = br = ones = None
        for _rep in range(reps):
            st0 = p1_state(0)
            for g in range(NG):
                p1_group(0, g, st0)
                if g == 0 and w2 is None:
                    w2, w2moe_raw, b2, wrt, br, ones = setup_tail()
            st1 = p1_state(1)
            for gg in range(4):
                p1_group(1, gg, st1, ptag="exp")
            rt0 = routing(0, st0[2])
            nxt = 4
            for g in range(NG):
                p2_group(0, g, st0, rt0)
                for _ in range(2):
                    if nxt < NG:
                        p1_group(1, nxt, st1, ptag="exp")
                        nxt += 1
                if g == 2:
                    rt1 = routing(1, st1[2])
            for g in range(NG):
                p2_group(1, g, st1, rt1)


def kernel(x, W_cv1, b_cv1, W_r, b_r, W_exp, b_exp, W_cv2, b_cv2):
    x = np.ascontiguousarray(np.asarray(x, dtype=np.float32))
    W_cv1 = np.asarray(W_cv1, dtype=np.float32)
    b_cv1 = np.asarray(b_cv1, dtype=np.float32)
    W_r = np.asarray(W_r, dtype=np.float32)
    b_r = np.asarray(b_r, dtype=np.float32)
    W_exp = np.asarray(W_exp, dtype=np.float32)
    b_exp = np.asarray(b_exp, dtype=np.float32)
    W_cv2 = np.asarray(W_cv2, dtype=np.float32)
    b_cv2 = np.asarray(b_cv2, dtype=np.float32)

    if "nc" not in _cache:
        _cache["nc"] = _build_program()
    nc = _cache["nc"]

    # host-side weight prep
    w1t = np.ascontiguousarray(W_cv1[:, :, 0, 0].T)  # [256 cin, 256 cout]
    w2t = np.ascontiguousarray(W_cv2[:, :, 0, 0].T)  # [384 cin, 256 cout]
    wrt = np.ascontiguousarray(W_r.T)  # [128, 4]
    # expert blob: [e, cin, ky, kx, cout] + bias column -> [E*C, 1153]
    wexp = np.empty((E, C, WBLOB + 1), dtype=np.float32)
    wexp[:, :, :WBLOB] = W_exp.transpose(0, 2, 3, 4, 1).reshape(E, C, WBLOB)
    # bias column: partition j holds b_exp[e, j] (j indexes cout when used as bias)
    wexp[:, :, WBLOB] = b_exp
    wexp = wexp.reshape(E * C, WBLOB + 1)

    shared = {
        "w1t": w1t,
        "b1": b_cv1.reshape(-1, 1),
        "wrt": wrt,
        "br": b_r.reshape(-1, 1),
        "wexp": wexp,
        "w2t": w2t,
        "b2": b_cv2.reshape(-1, 1),
    }
    xr = x.reshape(B, C1, S)
    in_maps = [
        {**shared, "x": np.ascontiguousarray(xr[i * BPC : (i + 1) * BPC])}
        for i in range(N_CORES)
    ]
    res = run_bass_kernel_spmd(nc, in_maps, core_ids=list(range(N_CORES)))
    _cache["last_results"] = res
    out = np.concatenate([res.results[i]["out"] for i in range(N_CORES)], axis=0)
    return out.reshape(B, 2 * C, H, W)
